# revision 25
# baseline (speedup 1.0000x reference)
"""CubeForwardProjection Trainium2 kernel.

Strategy: 16 (batch, segment) units over 8 NeuronCores, 2 segments per core
(both from the same batch). All data-dependent control (crop box, bilinear
coords, voxel assignment) is computed on host exactly mirroring the
reference's jnp float32 semantics and baked into per-core index/weight
tables. The device does all the heavy data-plane work per segment:

  1. dma_gather: fetch 4 bilinear-tap feature rows per output pixel from the
     pixel-major feature table in HBM (4096 rows x 512B).
  2. VectorE: weighted 4-tap reduction (crop+resize+mask folded into the tap
     weights) -> roiT (pixel-major 1024 x 128 ROI features).
  3. VectorE: scale rows by per-pixel scatter weight (validity/voxel-count
     folded) -> RsT.
  4. dma_scatter_add: accumulate RsT rows into the (32768, 128) voxel grid
     in HBM by host-computed voxel row index (scatter-mean numerator;
     1/count folded into the scale).
  5. TensorE transposes roiT -> channel-major roi2d, DMA'd out as the 2D ROI
     feature output.

The program is identical across cores (SPMD); all per-segment variation
lives in the table values.
"""
import numpy as np

for _p in ("/opt/trn_rl_repo",):
    import sys
    if _p not in sys.path:
        sys.path.insert(0, _p)

import jax
import jax.numpy as jnp

T = 32
HQ, WQ = 120, 160
NPIX = T * T          # 1024
C = 128
NCHUNK = 8            # 128-cell chunks
NSEG_PER_CORE = 2
NCORES = 8
NB, NS = 2, 8
V = T * T * T         # 32768
MAX_GATHER = 1024     # dma_gather num_idxs > 1024 crashes the exec unit

_CPU = jax.devices("cpu")[0]


# ----------------------------------------------------------------------------
# Host-side table construction (mirrors reference.py jnp f32 semantics)
# ----------------------------------------------------------------------------

def _crop_params(m_np):
    ys, xs = np.nonzero(m_np)
    y0, y1 = int(ys.min()), int(ys.max())
    x0, x1 = int(xs.min()), int(xs.max())
    side = max(y1 - y0, x1 - x0)
    cy, cx = (y0 + y1) // 2, (x0 + x1) // 2
    ny0 = max(cy - side // 2, 0)
    nx0 = max(cx - side // 2, 0)
    return ny0, nx0, side


def _bilinear_coords(n_in, size=T):
    x = (jnp.arange(size, dtype=jnp.float32) + 0.5) * (n_in / size) - 0.5
    x = jnp.clip(x, 0.0, n_in - 1.0)
    lo = jnp.floor(x).astype(jnp.int32)
    hi = jnp.minimum(lo + 1, n_in - 1)
    return np.asarray(lo), np.asarray(hi), np.asarray(x - lo)


def _nearest_coords(n_in, size=T):
    return np.asarray(jnp.floor(jnp.arange(size) * (n_in / size)).astype(jnp.int32))


def _project(depth, Kinv):
    H, W = depth.shape
    xs, ys = jnp.meshgrid(jnp.arange(W, dtype=depth.dtype),
                          jnp.arange(H, dtype=depth.dtype), indexing='xy')
    pix = jnp.stack([xs, ys, jnp.ones_like(xs)], axis=-1)
    cam = pix @ Kinv.T
    return cam / cam[..., 2:3] * depth[..., None]


def _segment_tables(points_np, m_np):
    """tapidx (1024,4) i32, tapw (1024,4) f32, sscale (1024,) f32,
    cellrow (1024,) i32 for one valid segment."""
    ny0, nx0, side = _crop_params(m_np)
    ylo, yhi, fy = _bilinear_coords(side)
    xlo, xhi, fx = _bilinear_coords(side)

    # pair taps: one gathered element = feat rows (y_a, x0) and (y_a, x0+1).
    # tapidx[q, a] = row of (y_a, x0); tapw[q, a*2+h] = weight of half h.
    tapidx = np.zeros((NPIX, 2), np.int32)
    tapw = np.zeros((NPIX, 4), np.float32)
    ys_abs = np.stack([ny0 + ylo, ny0 + yhi])
    wy = np.stack([1.0 - fy, fy]).astype(np.float32)
    wx = np.stack([1.0 - fx, fx]).astype(np.float32)
    x0_abs = nx0 + xlo
    x1_abs = nx0 + xhi
    m_at = lambda yy, xx: m_np[yy, xx].astype(np.float32)
    for a in range(2):
        for ty in range(T):
            y = int(ys_abs[a][ty])
            wya = wy[a][ty]
            if y >= HQ:
                continue
            for tx in range(T):
                q = ty * T + tx
                x0 = int(x0_abs[tx]); x1 = int(x1_abs[tx])
                w0 = w1 = 0.0
                idx = 0
                if x0 < WQ:
                    idx = y * WQ + x0
                    w0 = wya * wx[0][tx] * m_at(y, x0)
                    if x1 == x0:
                        w0 += wya * wx[1][tx] * m_at(y, x0)
                    elif x1 < WQ:
                        w1 = wya * wx[1][tx] * m_at(y, x1)
                tapidx[q, a] = idx
                tapw[q, a * 2 + 0] = w0
                tapw[q, a * 2 + 1] = w1

    # points path (mirror reference)
    yi = ny0 + _nearest_coords(side)
    xi = nx0 + _nearest_coords(side)
    mp = points_np * m_np[..., None].astype(points_np.dtype)
    ptile = np.zeros((T, T, 3), np.float32)
    oky, okx = yi < HQ, xi < WQ
    ptile[np.ix_(oky, okx)] = mp[yi[oky]][:, xi[okx]]
    ptsj = jnp.asarray(ptile.reshape(-1, 3))
    valid = jnp.all(ptsj != 0, axis=1)
    big = jnp.float32(1e30)
    mn = jnp.min(jnp.where(valid[:, None], ptsj, big), axis=0)
    mx = jnp.max(jnp.where(valid[:, None], ptsj, -big), axis=0)
    center = (mn + mx) * 0.5
    size = jnp.max(mx - mn)
    p = (ptsj - center) / size + 0.5
    inb = jnp.all((p >= 0) & (p <= 1), axis=1)
    w = np.asarray(valid & inb)
    gi = jnp.clip((p * (T - 1)).astype(jnp.int32), 0, T - 1)
    flat = np.asarray(gi[:, 0] * T * T + gi[:, 1] * T + gi[:, 2])

    act = np.nonzero(w)[0]
    cells, inv, cnt = np.unique(flat[act], return_inverse=True,
                                return_counts=True)
    sscale = np.zeros((NPIX,), np.float32)
    sscale[act] = 1.0 / cnt[inv].astype(np.float32)

    # scatter plan: cells sorted by count desc; per 128-cell chunk k the slot
    # length is the chunk's max count; cell (k,p) sums its pixels from slots.
    order = np.argsort(-cnt, kind='stable')
    n_u = len(cells)
    cellrow = np.full((NPIX,), -1, np.int32)       # -1 -> dummy (unoccupied)
    chunk_cnt = np.zeros((NCHUNK,), np.int32)      # per-chunk max count
    # slotpix[u_slot_position] lists: build per (chunk, partition)
    slot_lists = [[] for _ in range(NPIX)]         # per sorted-cell pixel lists
    pix_of_cell = [[] for _ in range(n_u)]
    for j in np.argsort(inv, kind='stable'):
        pix_of_cell[inv[j]].append(int(act[j]))
    for rank, ci in enumerate(order):
        cellrow[rank] = cells[ci]
        slot_lists[rank] = pix_of_cell[ci]
        k = rank // 128
        chunk_cnt[k] = max(chunk_cnt[k], cnt[ci])
    occupied = set(cells.tolist())
    dummy = next(i for i in range(NPIX + 1) if i not in occupied)
    cellrow[cellrow < 0] = dummy
    return tapidx, tapw, sscale, cellrow, slot_lists, chunk_cnt


def _empty_tables():
    return (np.zeros((NPIX, 2), np.int32), np.zeros((NPIX, 4), np.float32),
            np.zeros((NPIX,), np.float32), np.zeros((NPIX,), np.int32),
            [[] for _ in range(NPIX)], np.zeros((NCHUNK,), np.int32))


def _wrap16(idx_logical):
    """Logical gather order -> (128, n//16) int16 wrapped/replicated layout."""
    n = idx_logical.shape[0]
    a = idx_logical.reshape(n // 16, 16).T.astype(np.int16)   # (16, n//16)
    return np.ascontiguousarray(np.tile(a, (8, 1)))


def build_tables(encoder_features, depths, intrinsics, masks):
    """Returns (per-core input maps, Lk) where Lk[k] is the global slot
    length of cell chunk k."""
    ef = np.ascontiguousarray(np.asarray(encoder_features, dtype=np.float32))
    depths_np = np.asarray(depths, dtype=np.float32)
    masks_np = np.asarray(masks)
    with jax.default_device(_CPU):
        pts = {}
        for b in range(NB):
            K = jnp.asarray(np.asarray(intrinsics[b], dtype=np.float32))
            K = K.at[2, 2].multiply(4.0)
            Kinv = jnp.linalg.inv(K)
            pts[b] = np.asarray(_project(jnp.asarray(depths_np[b, ::4, ::4]),
                                         Kinv))
        segs = {}
        for b in range(NB):
            fd = depths_np[b, ::4, ::4]
            for s in range(NS):
                m = masks_np[b, s, ::4, ::4] & (fd > 0)
                if m.sum() < 10:
                    segs[(b, s)] = _empty_tables()
                else:
                    segs[(b, s)] = _segment_tables(pts[b], m)

    # global per-chunk slot lengths + active chunk count
    Lk = np.ones((NCHUNK,), np.int64)
    nch = 1
    for t in segs.values():
        Lk = np.maximum(Lk, np.asarray(t[5], np.int64))
        n_u = sum(len(x) > 0 for x in t[4])
        nch = max(nch, (n_u + 127) // 128)
    Lk = Lk[:nch]
    SL = int(Lk.sum())
    off = np.zeros((nch,), np.int64)
    off[1:] = np.cumsum(Lk)[:-1]

    in_maps = []
    for core in range(NCORES):
        b = core // 4
        s0 = 2 * (core % 4)
        featT = np.ascontiguousarray(ef[b].reshape(C, HQ * WQ).T)
        tapidx_w = np.zeros((NSEG_PER_CORE, 2, 128, 64), np.int16)
        tapw_t = np.zeros((NSEG_PER_CORE, 128, 32), np.float32)
        sscale_t = np.zeros((NSEG_PER_CORE, 128, 8), np.float32)
        cellidx_w = np.zeros((NSEG_PER_CORE, 128, nch * 8), np.int16)
        nslotg = (SL + 7) // 8
        slotidx_w = np.zeros((NSEG_PER_CORE, nslotg, 128, 64), np.int16)
        for kk in range(NSEG_PER_CORE):
            tapidx, tapw, sscale, cellrow, slot_lists, _ = segs[(b, s0 + kk)]
            # pair-gather token i2 = j2*128 + p ; j2 = ch*2 + a ; pixel = ch*128+p
            tl = np.zeros((2048,), np.int32)
            for j2 in range(16):
                ch, a = j2 // 2, j2 % 2
                tl[j2 * 128:(j2 + 1) * 128] = tapidx[ch * 128:(ch + 1) * 128, a]
            for g in range(2):
                tapidx_w[kk, g] = _wrap16(tl[g * 1024:(g + 1) * 1024])
            tapw_t[kk] = tapw.reshape(8, 128, 4).transpose(1, 0, 2).reshape(128, 32)
            sscale_t[kk] = sscale.reshape(8, 128).T
            # slot gather: slot s in [off[k], off[k]+Lk[k]) for chunk k;
            # token j = s*128 + p gathers stage row = pixel (or NPIX zero row)
            sl = np.full((SL * 128,), NPIX, np.int32)
            for rank in range(nch * 128):
                k, p = rank // 128, rank % 128
                for l, pix in enumerate(slot_lists[rank]):
                    sl[(off[k] + l) * 128 + p] = pix
            for g in range(nslotg):
                seg_tok = sl[g * 1024:(g + 1) * 1024]
                w = _wrap16(seg_tok)
                slotidx_w[kk, g, :, :w.shape[1]] = w
            # final unique-row scatter: token i -> cell row
            cellidx_w[kk] = _wrap16(cellrow[:nch * 128])
        featT_pad = np.vstack([featT, np.zeros((1, C), np.float32)])
        in_maps.append({
            "featT": featT_pad,
            "tapidx": tapidx_w,
            "tapw": tapw_t,
            "sscale": sscale_t,
            "slotidx": slotidx_w,
            "cellidx": cellidx_w,
            "ident": np.eye(128, dtype=np.float32),
        })
    return in_maps, [int(x) for x in Lk], nch


# ----------------------------------------------------------------------------
# Device program (SPMD, static)
# ----------------------------------------------------------------------------

def build_nc(Lk, nch=NCHUNK, ablate=()):
    import concourse.bass as bass
    import concourse.bacc as bacc
    import concourse.tile as tile
    from concourse import mybir

    A = set(ablate)  # {'taps','comb','scatter','f2'} stages to skip (perf study)

    f32 = mybir.dt.float32
    i16 = mybir.dt.int16

    SL = int(sum(Lk))
    off = [0] * nch
    for k in range(1, nch):
        off[k] = off[k - 1] + Lk[k - 1]
    nslotg = (SL + 7) // 8

    nc = bacc.Bacc(None, target_bir_lowering=False)
    featT = nc.declare_dram_parameter("featT", [HQ * WQ + 1, C], f32,
                                      isOutput=False)
    tapidx = nc.declare_dram_parameter("tapidx", [NSEG_PER_CORE, 2, 128, 64],
                                       i16, isOutput=False)
    tapw = nc.declare_dram_parameter("tapw", [NSEG_PER_CORE, 128, 32], f32,
                                     isOutput=False)
    sscale = nc.declare_dram_parameter("sscale", [NSEG_PER_CORE, 128, 8], f32,
                                       isOutput=False)
    slotidx = nc.declare_dram_parameter(
        "slotidx", [NSEG_PER_CORE, nslotg, 128, 64], i16, isOutput=False)
    cellidx = nc.declare_dram_parameter("cellidx", [NSEG_PER_CORE, 128, nch * 8],
                                        i16, isOutput=False)
    identp = nc.declare_dram_parameter("ident", [128, 128], f32, isOutput=False)
    f2out = nc.declare_dram_parameter("f2out", [NSEG_PER_CORE, C, NPIX], f32,
                                      isOutput=True)
    gouts = [nc.declare_dram_parameter(f"gout{k}", [V, C], f32, isOutput=True)
             for k in range(NSEG_PER_CORE)]

    with tile.TileContext(nc) as tc:
        with (
            tc.tile_pool(name="const", bufs=1) as cpool,
            tc.tile_pool(name="work", bufs=2) as pool,
            tc.tile_pool(name="psum", bufs=4, space="PSUM") as pp,
            tc.tile_pool(name="dram", bufs=1, space="DRAM") as dpool,
        ):
            stage0 = dpool.tile([NPIX + 1, C], f32, tag="stage0")
            stage1 = dpool.tile([NPIX + 1, C], f32, tag="stage1")
            stages = [stage0, stage1]
            ident = cpool.tile([128, 128], f32)
            nc.sync.dma_start(ident[:], identp[:])
            zrow = cpool.tile([1, C], f32)
            nc.vector.memset(zrow[:], 0.0)

            for k in range(NSEG_PER_CORE):
                if 'scatter' not in A: nc.sync.dma_start(stages[k][NPIX:NPIX + 1, :], zrow[:])

                tapw_t = pool.tile([128, 32], f32, tag="tapw")
                nc.sync.dma_start(tapw_t[:], tapw[k])
                sscale_t = pool.tile([128, 8], f32, tag="sscale")
                nc.sync.dma_start(sscale_t[:], sscale[k])
                cellidx_t = pool.tile([128, nch * 8], i16, tag="cellidx")
                nc.sync.dma_start(cellidx_t[:], cellidx[k])

                feat_pairs = bass.AP(featT[:].tensor, 0,
                                     [[C, HQ * WQ], [1, 2 * C]])
                taps = pool.tile([128, 32, 128], f32, tag="taps")
                tmp = pool.tile([128, 32, 128], f32, tag="tmp")
                roiT = pool.tile([128, 8, 128], f32, tag="roiT")
                for g in range(2 if 'taps' not in A else 0):
                    tapidx_t = pool.tile([128, 64], i16, tag="tapidx")
                    nc.sync.dma_start(tapidx_t[:], tapidx[k, g])
                    gout_ap = (taps[:, g * 16:(g + 1) * 16, :]
                               .rearrange("p a c -> p (a c)")
                               .rearrange("p (j c) -> p j c", c=2 * C))
                    nc.gpsimd.dma_gather(
                        gout_ap, feat_pairs, tapidx_t[:],
                        num_idxs=MAX_GATHER, num_idxs_reg=MAX_GATHER,
                        elem_size=2 * C, elem_step=C,
                    )
                    if 'comb' not in A:
                        sl16 = slice(g * 16, (g + 1) * 16)
                        nc.vector.tensor_tensor(
                            out=tmp[:, sl16, :], in0=taps[:, sl16, :],
                            in1=tapw_t[:, sl16].to_broadcast([128, 16, 128]),
                            op=mybir.AluOpType.mult,
                        )
                        nc.vector.tensor_reduce(
                            out=roiT[:, g * 4:(g + 1) * 4, :],
                            in_=tmp[:, sl16, :].rearrange(
                                "p (ch t) c -> p ch c t", ch=4, t=4),
                            axis=mybir.AxisListType.X,
                            op=mybir.AluOpType.add,
                        )
                rsT = pool.tile([128, 8, 128], f32, tag="rsT")
                if 'comb' not in A: nc.vector.tensor_tensor(
                    out=rsT[:], in0=roiT[:],
                    in1=sscale_t[:].to_broadcast([128, 8, 128]),
                    op=mybir.AluOpType.mult,
                )
                # stage pixel rows to DRAM (row q = pixel q), then slot-gather
                if 'scatter' not in A: nc.sync.dma_start(
                    stages[k][0:NPIX, :].rearrange("(ch p) c -> p ch c", p=128),
                    rsT[:],
                )
                slots = pool.tile([128, SL, 128], f32, tag="slots")
                for g in range(nslotg if 'scatter' not in A else 0):
                    ns = min(8, SL - g * 8)
                    sidx_t = pool.tile([128, 64], i16, tag="sidx")
                    nc.sync.dma_start(sidx_t[:], slotidx[k, g])
                    nc.gpsimd.dma_gather(
                        slots[:, g * 8:g * 8 + ns, :], stages[k][:, :],
                        sidx_t[:, :ns * 8],
                        num_idxs=ns * 128, num_idxs_reg=ns * 128, elem_size=C,
                    )
                gsum = pool.tile([128, nch, 128], f32, tag="gsum")
                for ch in range(nch if 'scatter' not in A else 0):
                    nc.vector.tensor_reduce(
                        out=gsum[:, ch, :],
                        in_=slots[:, off[ch]:off[ch] + Lk[ch], :]
                            .rearrange("p l c -> p c l"),
                        axis=mybir.AxisListType.X,
                        op=mybir.AluOpType.add,
                    )
                if 'scatter' not in A: nc.gpsimd.dma_scatter_add(
                    gouts[k][:], gsum[:], cellidx_t[:],
                    num_idxs=nch * 128, num_idxs_reg=nch * 128, elem_size=C,
                )
                roi2d = pool.tile([128, NPIX], f32, tag="roi2d")
                for ch in range(8 if 'f2' not in A else 0):
                    ps = pp.tile([128, 128], f32, tag="tps")
                    nc.tensor.transpose(out=ps[:], in_=roiT[:, ch, :],
                                        identity=ident[:])
                    nc.scalar.copy(out=roi2d[:, ch * 128:(ch + 1) * 128],
                                   in_=ps[:])
                if 'f2' not in A: nc.sync.dma_start(f2out[k], roi2d[:])
    nc.finalize()
    return nc


_NC_CACHE = {}


def _get_nc(Lk, nch):
    key = (tuple(Lk), nch)
    if key not in _NC_CACHE:
        _NC_CACHE[key] = build_nc(Lk, nch)
    return _NC_CACHE[key]


# ----------------------------------------------------------------------------
# Entry point
# ----------------------------------------------------------------------------

def kernel(encoder_features, depths, intrinsics, masks, _trace=False):
    from concourse.bass_utils import run_bass_kernel_spmd

    in_maps, Lk, nch = build_tables(encoder_features, depths, intrinsics, masks)
    nc = _get_nc(Lk, nch)
    res = run_bass_kernel_spmd(nc, in_maps, core_ids=list(range(NCORES)),
                               trace=_trace)
    f2 = np.zeros((NB, NS, C, T, T), np.float32)
    g = np.zeros((NB, NS, T, T, T, C), np.float32)
    for core in range(NCORES):
        b = core // 4
        s0 = 2 * (core % 4)
        r = res.results[core]
        f2[b, s0:s0 + 2] = r["f2out"].reshape(NSEG_PER_CORE, C, T, T)
        for k in range(NSEG_PER_CORE):
            g[b, s0 + k] = r[f"gout{k}"].reshape(T, T, T, C)
    if _trace:
        kernel._last_result = res
    return f2, g


# revision 27
# speedup vs baseline: 1.0414x; 1.0414x over previous
"""CubeForwardProjection Trainium2 kernel.

Strategy: 16 (batch, segment) units over 8 NeuronCores, 2 segments per core
(both from the same batch). All data-dependent control (crop box, bilinear
coords, voxel assignment) is computed on host exactly mirroring the
reference's jnp float32 semantics and baked into per-core index/weight
tables. The device does all the heavy data-plane work per segment:

  1. dma_gather: fetch 4 bilinear-tap feature rows per output pixel from the
     pixel-major feature table in HBM (4096 rows x 512B).
  2. VectorE: weighted 4-tap reduction (crop+resize+mask folded into the tap
     weights) -> roiT (pixel-major 1024 x 128 ROI features).
  3. VectorE: scale rows by per-pixel scatter weight (validity/voxel-count
     folded) -> RsT.
  4. dma_scatter_add: accumulate RsT rows into the (32768, 128) voxel grid
     in HBM by host-computed voxel row index (scatter-mean numerator;
     1/count folded into the scale).
  5. TensorE transposes roiT -> channel-major roi2d, DMA'd out as the 2D ROI
     feature output.

The program is identical across cores (SPMD); all per-segment variation
lives in the table values.
"""
import numpy as np

for _p in ("/opt/trn_rl_repo",):
    import sys
    if _p not in sys.path:
        sys.path.insert(0, _p)

import jax
import jax.numpy as jnp

T = 32
HQ, WQ = 120, 160
NPIX = T * T          # 1024
C = 128
NCHUNK = 8            # 128-cell chunks
NSEG_PER_CORE = 2
NCORES = 8
NB, NS = 2, 8
V = T * T * T         # 32768
MAX_GATHER = 1024     # dma_gather num_idxs > 1024 crashes the exec unit
WORK_BUFS = 3         # working tile pool slots (overlap depth)

_CPU = jax.devices("cpu")[0]


# ----------------------------------------------------------------------------
# Host-side table construction (mirrors reference.py jnp f32 semantics)
# ----------------------------------------------------------------------------

def _crop_params(m_np):
    ys, xs = np.nonzero(m_np)
    y0, y1 = int(ys.min()), int(ys.max())
    x0, x1 = int(xs.min()), int(xs.max())
    side = max(y1 - y0, x1 - x0)
    cy, cx = (y0 + y1) // 2, (x0 + x1) // 2
    ny0 = max(cy - side // 2, 0)
    nx0 = max(cx - side // 2, 0)
    return ny0, nx0, side


def _bilinear_coords(n_in, size=T):
    x = (jnp.arange(size, dtype=jnp.float32) + 0.5) * (n_in / size) - 0.5
    x = jnp.clip(x, 0.0, n_in - 1.0)
    lo = jnp.floor(x).astype(jnp.int32)
    hi = jnp.minimum(lo + 1, n_in - 1)
    return np.asarray(lo), np.asarray(hi), np.asarray(x - lo)


def _nearest_coords(n_in, size=T):
    return np.asarray(jnp.floor(jnp.arange(size) * (n_in / size)).astype(jnp.int32))


def _project(depth, Kinv):
    H, W = depth.shape
    xs, ys = jnp.meshgrid(jnp.arange(W, dtype=depth.dtype),
                          jnp.arange(H, dtype=depth.dtype), indexing='xy')
    pix = jnp.stack([xs, ys, jnp.ones_like(xs)], axis=-1)
    cam = pix @ Kinv.T
    return cam / cam[..., 2:3] * depth[..., None]


def _segment_tables(points_np, m_np):
    """tapidx (1024,4) i32, tapw (1024,4) f32, sscale (1024,) f32,
    cellrow (1024,) i32 for one valid segment."""
    ny0, nx0, side = _crop_params(m_np)
    ylo, yhi, fy = _bilinear_coords(side)
    xlo, xhi, fx = _bilinear_coords(side)

    # pair taps: one gathered element = feat rows (y_a, x0) and (y_a, x0+1).
    # tapidx[q, a] = row of (y_a, x0); tapw[q, a*2+h] = weight of half h.
    tapidx = np.zeros((NPIX, 2), np.int32)
    tapw = np.zeros((NPIX, 4), np.float32)
    ys_abs = np.stack([ny0 + ylo, ny0 + yhi])
    wy = np.stack([1.0 - fy, fy]).astype(np.float32)
    wx = np.stack([1.0 - fx, fx]).astype(np.float32)
    x0_abs = nx0 + xlo
    x1_abs = nx0 + xhi
    m_at = lambda yy, xx: m_np[yy, xx].astype(np.float32)
    for a in range(2):
        for ty in range(T):
            y = int(ys_abs[a][ty])
            wya = wy[a][ty]
            if y >= HQ:
                continue
            for tx in range(T):
                q = ty * T + tx
                x0 = int(x0_abs[tx]); x1 = int(x1_abs[tx])
                w0 = w1 = 0.0
                idx = 0
                if x0 < WQ:
                    idx = y * WQ + x0
                    w0 = wya * wx[0][tx] * m_at(y, x0)
                    if x1 == x0:
                        w0 += wya * wx[1][tx] * m_at(y, x0)
                    elif x1 < WQ:
                        w1 = wya * wx[1][tx] * m_at(y, x1)
                tapidx[q, a] = idx
                tapw[q, a * 2 + 0] = w0
                tapw[q, a * 2 + 1] = w1

    # points path (mirror reference)
    yi = ny0 + _nearest_coords(side)
    xi = nx0 + _nearest_coords(side)
    mp = points_np * m_np[..., None].astype(points_np.dtype)
    ptile = np.zeros((T, T, 3), np.float32)
    oky, okx = yi < HQ, xi < WQ
    ptile[np.ix_(oky, okx)] = mp[yi[oky]][:, xi[okx]]
    ptsj = jnp.asarray(ptile.reshape(-1, 3))
    valid = jnp.all(ptsj != 0, axis=1)
    big = jnp.float32(1e30)
    mn = jnp.min(jnp.where(valid[:, None], ptsj, big), axis=0)
    mx = jnp.max(jnp.where(valid[:, None], ptsj, -big), axis=0)
    center = (mn + mx) * 0.5
    size = jnp.max(mx - mn)
    p = (ptsj - center) / size + 0.5
    inb = jnp.all((p >= 0) & (p <= 1), axis=1)
    w = np.asarray(valid & inb)
    gi = jnp.clip((p * (T - 1)).astype(jnp.int32), 0, T - 1)
    flat = np.asarray(gi[:, 0] * T * T + gi[:, 1] * T + gi[:, 2])

    act = np.nonzero(w)[0]
    cells, inv, cnt = np.unique(flat[act], return_inverse=True,
                                return_counts=True)
    sscale = np.zeros((NPIX,), np.float32)
    sscale[act] = 1.0 / cnt[inv].astype(np.float32)

    # scatter plan: cells sorted by count desc; per 128-cell chunk k the slot
    # length is the chunk's max count; cell (k,p) sums its pixels from slots.
    order = np.argsort(-cnt, kind='stable')
    n_u = len(cells)
    cellrow = np.full((NPIX,), -1, np.int32)       # -1 -> dummy (unoccupied)
    chunk_cnt = np.zeros((NCHUNK,), np.int32)      # per-chunk max count
    # slotpix[u_slot_position] lists: build per (chunk, partition)
    slot_lists = [[] for _ in range(NPIX)]         # per sorted-cell pixel lists
    pix_of_cell = [[] for _ in range(n_u)]
    for j in np.argsort(inv, kind='stable'):
        pix_of_cell[inv[j]].append(int(act[j]))
    for rank, ci in enumerate(order):
        cellrow[rank] = cells[ci]
        slot_lists[rank] = pix_of_cell[ci]
        k = rank // 128
        chunk_cnt[k] = max(chunk_cnt[k], cnt[ci])
    occupied = set(cells.tolist())
    dummy = next(i for i in range(NPIX + 1) if i not in occupied)
    cellrow[cellrow < 0] = dummy
    return tapidx, tapw, sscale, cellrow, slot_lists, chunk_cnt


def _empty_tables():
    return (np.zeros((NPIX, 2), np.int32), np.zeros((NPIX, 4), np.float32),
            np.zeros((NPIX,), np.float32), np.zeros((NPIX,), np.int32),
            [[] for _ in range(NPIX)], np.zeros((NCHUNK,), np.int32))


def _wrap16(idx_logical):
    """Logical gather order -> (128, n//16) int16 wrapped/replicated layout."""
    n = idx_logical.shape[0]
    a = idx_logical.reshape(n // 16, 16).T.astype(np.int16)   # (16, n//16)
    return np.ascontiguousarray(np.tile(a, (8, 1)))


def build_tables(encoder_features, depths, intrinsics, masks):
    """Returns (per-core input maps, Lk) where Lk[k] is the global slot
    length of cell chunk k."""
    ef = np.ascontiguousarray(np.asarray(encoder_features, dtype=np.float32))
    depths_np = np.asarray(depths, dtype=np.float32)
    masks_np = np.asarray(masks)
    with jax.default_device(_CPU):
        pts = {}
        for b in range(NB):
            K = jnp.asarray(np.asarray(intrinsics[b], dtype=np.float32))
            K = K.at[2, 2].multiply(4.0)
            Kinv = jnp.linalg.inv(K)
            pts[b] = np.asarray(_project(jnp.asarray(depths_np[b, ::4, ::4]),
                                         Kinv))
        segs = {}
        for b in range(NB):
            fd = depths_np[b, ::4, ::4]
            for s in range(NS):
                m = masks_np[b, s, ::4, ::4] & (fd > 0)
                if m.sum() < 10:
                    segs[(b, s)] = _empty_tables()
                else:
                    segs[(b, s)] = _segment_tables(pts[b], m)

    # global per-chunk slot lengths + active chunk count
    Lk = np.ones((NCHUNK,), np.int64)
    nch = 1
    for t in segs.values():
        Lk = np.maximum(Lk, np.asarray(t[5], np.int64))
        n_u = sum(len(x) > 0 for x in t[4])
        nch = max(nch, (n_u + 127) // 128)
    Lk = Lk[:nch]
    SL = int(Lk.sum())
    off = np.zeros((nch,), np.int64)
    off[1:] = np.cumsum(Lk)[:-1]

    in_maps = []
    for core in range(NCORES):
        b = core // 4
        s0 = 2 * (core % 4)
        featT = np.ascontiguousarray(ef[b].reshape(C, HQ * WQ).T)
        tapidx_w = np.zeros((NSEG_PER_CORE, 2, 128, 64), np.int16)
        tapw_t = np.zeros((NSEG_PER_CORE, 128, 32), np.float32)
        sscale_t = np.zeros((NSEG_PER_CORE, 128, 8), np.float32)
        cellidx_w = np.zeros((NSEG_PER_CORE, 128, nch * 8), np.int16)
        nslotg = (SL + 7) // 8
        slotidx_w = np.zeros((NSEG_PER_CORE, nslotg, 128, 64), np.int16)
        for kk in range(NSEG_PER_CORE):
            tapidx, tapw, sscale, cellrow, slot_lists, _ = segs[(b, s0 + kk)]
            # pair-gather token i2 = j2*128 + p ; j2 = ch*2 + a ; pixel = ch*128+p
            tl = np.zeros((2048,), np.int32)
            for j2 in range(16):
                ch, a = j2 // 2, j2 % 2
                tl[j2 * 128:(j2 + 1) * 128] = tapidx[ch * 128:(ch + 1) * 128, a]
            for g in range(2):
                tapidx_w[kk, g] = _wrap16(tl[g * 1024:(g + 1) * 1024])
            tapw_t[kk] = tapw.reshape(8, 128, 4).transpose(1, 0, 2).reshape(128, 32)
            sscale_t[kk] = sscale.reshape(8, 128).T
            # slot gather: slot s in [off[k], off[k]+Lk[k]) for chunk k;
            # token j = s*128 + p gathers stage row = pixel (or NPIX zero row)
            sl = np.full((SL * 128,), NPIX, np.int32)
            for rank in range(nch * 128):
                k, p = rank // 128, rank % 128
                for l, pix in enumerate(slot_lists[rank]):
                    sl[(off[k] + l) * 128 + p] = pix
            for g in range(nslotg):
                seg_tok = sl[g * 1024:(g + 1) * 1024]
                w = _wrap16(seg_tok)
                slotidx_w[kk, g, :, :w.shape[1]] = w
            # final unique-row scatter: token i -> cell row
            cellidx_w[kk] = _wrap16(cellrow[:nch * 128])
        featT_pad = np.vstack([featT, np.zeros((1, C), np.float32)])
        in_maps.append({
            "featT": featT_pad,
            "tapidx": tapidx_w,
            "tapw": tapw_t,
            "sscale": sscale_t,
            "slotidx": slotidx_w,
            "cellidx": cellidx_w,
            "ident": np.eye(128, dtype=np.float32),
        })
    return in_maps, [int(x) for x in Lk], nch


# ----------------------------------------------------------------------------
# Device program (SPMD, static)
# ----------------------------------------------------------------------------

def build_nc(Lk, nch=NCHUNK, ablate=()):
    import concourse.bass as bass
    import concourse.bacc as bacc
    import concourse.tile as tile
    from concourse import mybir

    A = set(ablate)  # {'taps','comb','scatter','f2'} stages to skip (perf study)

    f32 = mybir.dt.float32
    i16 = mybir.dt.int16

    SL = int(sum(Lk))
    off = [0] * nch
    for k in range(1, nch):
        off[k] = off[k - 1] + Lk[k - 1]
    nslotg = (SL + 7) // 8

    nc = bacc.Bacc(None, target_bir_lowering=False)
    featT = nc.declare_dram_parameter("featT", [HQ * WQ + 1, C], f32,
                                      isOutput=False)
    tapidx = nc.declare_dram_parameter("tapidx", [NSEG_PER_CORE, 2, 128, 64],
                                       i16, isOutput=False)
    tapw = nc.declare_dram_parameter("tapw", [NSEG_PER_CORE, 128, 32], f32,
                                     isOutput=False)
    sscale = nc.declare_dram_parameter("sscale", [NSEG_PER_CORE, 128, 8], f32,
                                       isOutput=False)
    slotidx = nc.declare_dram_parameter(
        "slotidx", [NSEG_PER_CORE, nslotg, 128, 64], i16, isOutput=False)
    cellidx = nc.declare_dram_parameter("cellidx", [NSEG_PER_CORE, 128, nch * 8],
                                        i16, isOutput=False)
    identp = nc.declare_dram_parameter("ident", [128, 128], f32, isOutput=False)
    f2out = nc.declare_dram_parameter("f2out", [NSEG_PER_CORE, C, NPIX], f32,
                                      isOutput=True)
    gouts = [nc.declare_dram_parameter(f"gout{k}", [V, C], f32, isOutput=True)
             for k in range(NSEG_PER_CORE)]

    with tile.TileContext(nc) as tc:
        with (
            tc.tile_pool(name="const", bufs=1) as cpool,
            tc.tile_pool(name="work", bufs=WORK_BUFS) as pool,
            tc.tile_pool(name="psum", bufs=4, space="PSUM") as pp,
            tc.tile_pool(name="dram", bufs=1, space="DRAM") as dpool,
        ):
            stage0 = dpool.tile([NPIX + 1, C], f32, tag="stage0")
            stage1 = dpool.tile([NPIX + 1, C], f32, tag="stage1")
            stages = [stage0, stage1]
            ident = cpool.tile([128, 128], f32)
            nc.sync.dma_start(ident[:], identp[:])
            zrow = cpool.tile([1, C], f32)
            nc.vector.memset(zrow[:], 0.0)

            for k in range(NSEG_PER_CORE):
                if 'scatter' not in A: nc.sync.dma_start(stages[k][NPIX:NPIX + 1, :], zrow[:])

                tapw_t = pool.tile([128, 32], f32, tag="tapw")
                nc.sync.dma_start(tapw_t[:], tapw[k])
                sscale_t = pool.tile([128, 8], f32, tag="sscale")
                nc.sync.dma_start(sscale_t[:], sscale[k])
                cellidx_t = pool.tile([128, nch * 8], i16, tag="cellidx")
                nc.sync.dma_start(cellidx_t[:], cellidx[k])

                feat_pairs = bass.AP(featT[:].tensor, 0,
                                     [[C, HQ * WQ], [1, 2 * C]])
                taps = pool.tile([128, 32, 128], f32, tag="taps")
                tmp = pool.tile([128, 32, 128], f32, tag="tmp")
                roiT = pool.tile([128, 8, 128], f32, tag="roiT")
                for g in range(2 if 'taps' not in A else 0):
                    tapidx_t = pool.tile([128, 64], i16, tag="tapidx")
                    nc.sync.dma_start(tapidx_t[:], tapidx[k, g])
                    gout_ap = (taps[:, g * 16:(g + 1) * 16, :]
                               .rearrange("p a c -> p (a c)")
                               .rearrange("p (j c) -> p j c", c=2 * C))
                    nc.gpsimd.dma_gather(
                        gout_ap, feat_pairs, tapidx_t[:],
                        num_idxs=MAX_GATHER, num_idxs_reg=MAX_GATHER,
                        elem_size=2 * C, elem_step=C,
                    )
                    if 'comb' not in A:
                        sl16 = slice(g * 16, (g + 1) * 16)
                        nc.vector.tensor_tensor(
                            out=tmp[:, sl16, :], in0=taps[:, sl16, :],
                            in1=tapw_t[:, sl16].to_broadcast([128, 16, 128]),
                            op=mybir.AluOpType.mult,
                        )
                        nc.vector.tensor_reduce(
                            out=roiT[:, g * 4:(g + 1) * 4, :],
                            in_=tmp[:, sl16, :].rearrange(
                                "p (ch t) c -> p ch c t", ch=4, t=4),
                            axis=mybir.AxisListType.X,
                            op=mybir.AluOpType.add,
                        )
                rsT = pool.tile([128, 8, 128], f32, tag="rsT")
                if 'comb' not in A: nc.vector.tensor_tensor(
                    out=rsT[:], in0=roiT[:],
                    in1=sscale_t[:].to_broadcast([128, 8, 128]),
                    op=mybir.AluOpType.mult,
                )
                # stage pixel rows to DRAM (row q = pixel q), then slot-gather
                if 'scatter' not in A: nc.sync.dma_start(
                    stages[k][0:NPIX, :].rearrange("(ch p) c -> p ch c", p=128),
                    rsT[:],
                )
                slots = pool.tile([128, SL, 128], f32, tag="slots")
                for g in range(nslotg if 'scatter' not in A else 0):
                    ns = min(8, SL - g * 8)
                    sidx_t = pool.tile([128, 64], i16, tag="sidx")
                    nc.sync.dma_start(sidx_t[:], slotidx[k, g])
                    nc.gpsimd.dma_gather(
                        slots[:, g * 8:g * 8 + ns, :], stages[k][:, :],
                        sidx_t[:, :ns * 8],
                        num_idxs=ns * 128, num_idxs_reg=ns * 128, elem_size=C,
                    )
                gsum = pool.tile([128, nch, 128], f32, tag="gsum")
                for ch in range(nch if 'scatter' not in A else 0):
                    nc.vector.tensor_reduce(
                        out=gsum[:, ch, :],
                        in_=slots[:, off[ch]:off[ch] + Lk[ch], :]
                            .rearrange("p l c -> p c l"),
                        axis=mybir.AxisListType.X,
                        op=mybir.AluOpType.add,
                    )
                if 'scatter' not in A: nc.gpsimd.dma_scatter_add(
                    gouts[k][:], gsum[:], cellidx_t[:],
                    num_idxs=nch * 128, num_idxs_reg=nch * 128, elem_size=C,
                )
                roi2d = pool.tile([128, NPIX], f32, tag="roi2d")
                for ch in range(8 if 'f2' not in A else 0):
                    ps = pp.tile([128, 128], f32, tag="tps")
                    nc.tensor.transpose(out=ps[:], in_=roiT[:, ch, :],
                                        identity=ident[:])
                    nc.scalar.copy(out=roi2d[:, ch * 128:(ch + 1) * 128],
                                   in_=ps[:])
                if 'f2' not in A: nc.sync.dma_start(f2out[k], roi2d[:])
    nc.finalize()
    return nc


_NC_CACHE = {}


def _get_nc(Lk, nch):
    key = (tuple(Lk), nch)
    if key not in _NC_CACHE:
        _NC_CACHE[key] = build_nc(Lk, nch)
    return _NC_CACHE[key]


# ----------------------------------------------------------------------------
# Entry point
# ----------------------------------------------------------------------------

def kernel(encoder_features, depths, intrinsics, masks, _trace=False):
    from concourse.bass_utils import run_bass_kernel_spmd

    in_maps, Lk, nch = build_tables(encoder_features, depths, intrinsics, masks)
    nc = _get_nc(Lk, nch)
    res = run_bass_kernel_spmd(nc, in_maps, core_ids=list(range(NCORES)),
                               trace=_trace)
    f2 = np.zeros((NB, NS, C, T, T), np.float32)
    g = np.zeros((NB, NS, T, T, T, C), np.float32)
    for core in range(NCORES):
        b = core // 4
        s0 = 2 * (core % 4)
        r = res.results[core]
        f2[b, s0:s0 + 2] = r["f2out"].reshape(NSEG_PER_CORE, C, T, T)
        for k in range(NSEG_PER_CORE):
            g[b, s0 + k] = r[f"gout{k}"].reshape(T, T, T, C)
    if _trace:
        kernel._last_result = res
    return f2, g


# revision 33
# speedup vs baseline: 1.0896x; 1.0462x over previous
"""CubeForwardProjection Trainium2 kernel.

Strategy: 16 (batch, segment) units over 8 NeuronCores, 2 segments per core
(both from the same batch). All data-dependent control (crop box, bilinear
coords, voxel assignment) is computed on host exactly mirroring the
reference's jnp float32 semantics and baked into per-core index/weight
tables. The device does all the heavy data-plane work per segment:

  1. dma_gather (2 x 1024 idx): fetch the 4 bilinear taps per output pixel
     as 2 adjacent-row PAIRS (overlapping 1KB elements, elem_step=128) from
     the pixel-major feature table in HBM.
  2. VectorE: weighted 4-tap reduction (crop+resize+mask all folded into the
     tap weights) -> roiT (pixel-major 1024 x 128 ROI features), sliced per
     gather group for pipelining.
  3. VectorE: scale rows by per-pixel scatter weight w/cnt (validity and
     scatter-mean divisor folded) -> RsT; staged to DRAM pixel-row-major.
  4. dma_gather slots: regroup pixel rows by voxel cell (cells sorted by
     count desc; per-128-cell chunk slot length = chunk max count), then
     VectorE segmented reduce -> per-cell sums.
  5. dma_scatter_add: one pass of UNIQUE cell rows into the (32768, 128)
     voxel grid (duplicate-row scatter_add loses updates - verified on HW -
     so duplicates are pre-combined in step 4; dummy rows carry zeros).
  6. TensorE transposes roiT -> channel-major roi2d, DMA'd out as the 2D ROI
     feature output.

The program is identical across cores (SPMD); all per-segment variation
lives in the table values; only the slot lengths Lk / active chunk count
(data-dependent, known at build time) parameterize the compiled shape.

Known HW constraints found while bringing this up (cost one device crash
each): dma_gather/dma_scatter_add num_idxs must be <= 1024, and
dma_scatter_add does NOT accumulate correctly across duplicate indices
within one instruction (RMW race between DMA engines).
"""
import numpy as np

for _p in ("/opt/trn_rl_repo",):
    import sys
    if _p not in sys.path:
        sys.path.insert(0, _p)

import jax
import jax.numpy as jnp

T = 32
HQ, WQ = 120, 160
NPIX = T * T          # 1024
C = 128
NCHUNK = 8            # 128-cell chunks
NSEG_PER_CORE = 2
NCORES = 8
NB, NS = 2, 8
V = T * T * T         # 32768
MAX_GATHER = 1024     # dma_gather num_idxs > 1024 crashes the exec unit
WORK_BUFS = 2         # working tile pool slots (overlap depth)

_CPU = jax.devices("cpu")[0]


# ----------------------------------------------------------------------------
# Host-side table construction (mirrors reference.py jnp f32 semantics)
# ----------------------------------------------------------------------------

def _crop_params(m_np):
    ys, xs = np.nonzero(m_np)
    y0, y1 = int(ys.min()), int(ys.max())
    x0, x1 = int(xs.min()), int(xs.max())
    side = max(y1 - y0, x1 - x0)
    cy, cx = (y0 + y1) // 2, (x0 + x1) // 2
    ny0 = max(cy - side // 2, 0)
    nx0 = max(cx - side // 2, 0)
    return ny0, nx0, side


def _bilinear_coords(n_in, size=T):
    x = (jnp.arange(size, dtype=jnp.float32) + 0.5) * (n_in / size) - 0.5
    x = jnp.clip(x, 0.0, n_in - 1.0)
    lo = jnp.floor(x).astype(jnp.int32)
    hi = jnp.minimum(lo + 1, n_in - 1)
    return np.asarray(lo), np.asarray(hi), np.asarray(x - lo)


def _nearest_coords(n_in, size=T):
    return np.asarray(jnp.floor(jnp.arange(size) * (n_in / size)).astype(jnp.int32))


def _project(depth, Kinv):
    H, W = depth.shape
    xs, ys = jnp.meshgrid(jnp.arange(W, dtype=depth.dtype),
                          jnp.arange(H, dtype=depth.dtype), indexing='xy')
    pix = jnp.stack([xs, ys, jnp.ones_like(xs)], axis=-1)
    cam = pix @ Kinv.T
    return cam / cam[..., 2:3] * depth[..., None]


def _segment_tables(points_np, m_np):
    """tapidx (1024,4) i32, tapw (1024,4) f32, sscale (1024,) f32,
    cellrow (1024,) i32 for one valid segment."""
    ny0, nx0, side = _crop_params(m_np)
    ylo, yhi, fy = _bilinear_coords(side)
    xlo, xhi, fx = _bilinear_coords(side)

    # pair taps: one gathered element = feat rows (y_a, x0) and (y_a, x0+1).
    # tapidx[q, a] = row of (y_a, x0); tapw[q, a*2+h] = weight of half h.
    tapidx = np.zeros((NPIX, 2), np.int32)
    tapw = np.zeros((NPIX, 4), np.float32)
    ys_abs = np.stack([ny0 + ylo, ny0 + yhi])
    wy = np.stack([1.0 - fy, fy]).astype(np.float32)
    wx = np.stack([1.0 - fx, fx]).astype(np.float32)
    x0_abs = nx0 + xlo
    x1_abs = nx0 + xhi
    m_at = lambda yy, xx: m_np[yy, xx].astype(np.float32)
    for a in range(2):
        for ty in range(T):
            y = int(ys_abs[a][ty])
            wya = wy[a][ty]
            if y >= HQ:
                continue
            for tx in range(T):
                q = ty * T + tx
                x0 = int(x0_abs[tx]); x1 = int(x1_abs[tx])
                w0 = w1 = 0.0
                idx = 0
                if x0 < WQ:
                    idx = y * WQ + x0
                    w0 = wya * wx[0][tx] * m_at(y, x0)
                    if x1 == x0:
                        w0 += wya * wx[1][tx] * m_at(y, x0)
                    elif x1 < WQ:
                        w1 = wya * wx[1][tx] * m_at(y, x1)
                tapidx[q, a] = idx
                tapw[q, a * 2 + 0] = w0
                tapw[q, a * 2 + 1] = w1

    # points path (mirror reference)
    yi = ny0 + _nearest_coords(side)
    xi = nx0 + _nearest_coords(side)
    mp = points_np * m_np[..., None].astype(points_np.dtype)
    ptile = np.zeros((T, T, 3), np.float32)
    oky, okx = yi < HQ, xi < WQ
    ptile[np.ix_(oky, okx)] = mp[yi[oky]][:, xi[okx]]
    ptsj = jnp.asarray(ptile.reshape(-1, 3))
    valid = jnp.all(ptsj != 0, axis=1)
    big = jnp.float32(1e30)
    mn = jnp.min(jnp.where(valid[:, None], ptsj, big), axis=0)
    mx = jnp.max(jnp.where(valid[:, None], ptsj, -big), axis=0)
    center = (mn + mx) * 0.5
    size = jnp.max(mx - mn)
    p = (ptsj - center) / size + 0.5
    inb = jnp.all((p >= 0) & (p <= 1), axis=1)
    w = np.asarray(valid & inb)
    gi = jnp.clip((p * (T - 1)).astype(jnp.int32), 0, T - 1)
    flat = np.asarray(gi[:, 0] * T * T + gi[:, 1] * T + gi[:, 2])

    act = np.nonzero(w)[0]
    cells, inv, cnt = np.unique(flat[act], return_inverse=True,
                                return_counts=True)
    sscale = np.zeros((NPIX,), np.float32)
    sscale[act] = 1.0 / cnt[inv].astype(np.float32)

    # scatter plan: cells sorted by count desc; per 128-cell chunk k the slot
    # length is the chunk's max count; cell (k,p) sums its pixels from slots.
    order = np.argsort(-cnt, kind='stable')
    n_u = len(cells)
    cellrow = np.full((NPIX,), -1, np.int32)       # -1 -> dummy (unoccupied)
    chunk_cnt = np.zeros((NCHUNK,), np.int32)      # per-chunk max count
    # slotpix[u_slot_position] lists: build per (chunk, partition)
    slot_lists = [[] for _ in range(NPIX)]         # per sorted-cell pixel lists
    pix_of_cell = [[] for _ in range(n_u)]
    for j in np.argsort(inv, kind='stable'):
        pix_of_cell[inv[j]].append(int(act[j]))
    for rank, ci in enumerate(order):
        cellrow[rank] = cells[ci]
        slot_lists[rank] = pix_of_cell[ci]
        k = rank // 128
        chunk_cnt[k] = max(chunk_cnt[k], cnt[ci])
    occupied = set(cells.tolist())
    dummy = next(i for i in range(NPIX + 1) if i not in occupied)
    cellrow[cellrow < 0] = dummy
    return tapidx, tapw, sscale, cellrow, slot_lists, chunk_cnt


def _empty_tables():
    return (np.zeros((NPIX, 2), np.int32), np.zeros((NPIX, 4), np.float32),
            np.zeros((NPIX,), np.float32), np.zeros((NPIX,), np.int32),
            [[] for _ in range(NPIX)], np.zeros((NCHUNK,), np.int32))


def _wrap16(idx_logical):
    """Logical gather order -> (128, n//16) int16 wrapped/replicated layout."""
    n = idx_logical.shape[0]
    a = idx_logical.reshape(n // 16, 16).T.astype(np.int16)   # (16, n//16)
    return np.ascontiguousarray(np.tile(a, (8, 1)))


def build_tables(encoder_features, depths, intrinsics, masks):
    """Returns (per-core input maps, Lk) where Lk[k] is the global slot
    length of cell chunk k."""
    ef = np.ascontiguousarray(np.asarray(encoder_features, dtype=np.float32))
    depths_np = np.asarray(depths, dtype=np.float32)
    masks_np = np.asarray(masks)
    with jax.default_device(_CPU):
        pts = {}
        for b in range(NB):
            K = jnp.asarray(np.asarray(intrinsics[b], dtype=np.float32))
            K = K.at[2, 2].multiply(4.0)
            Kinv = jnp.linalg.inv(K)
            pts[b] = np.asarray(_project(jnp.asarray(depths_np[b, ::4, ::4]),
                                         Kinv))
        segs = {}
        for b in range(NB):
            fd = depths_np[b, ::4, ::4]
            for s in range(NS):
                m = masks_np[b, s, ::4, ::4] & (fd > 0)
                if m.sum() < 10:
                    segs[(b, s)] = _empty_tables()
                else:
                    segs[(b, s)] = _segment_tables(pts[b], m)

    # global per-chunk slot lengths + active chunk count
    Lk = np.ones((NCHUNK,), np.int64)
    nch = 1
    for t in segs.values():
        Lk = np.maximum(Lk, np.asarray(t[5], np.int64))
        n_u = sum(len(x) > 0 for x in t[4])
        nch = max(nch, (n_u + 127) // 128)
    Lk = Lk[:nch]
    SL = int(Lk.sum())
    off = np.zeros((nch,), np.int64)
    off[1:] = np.cumsum(Lk)[:-1]

    in_maps = []
    for core in range(NCORES):
        b = core // 4
        s0 = 2 * (core % 4)
        featT = np.ascontiguousarray(ef[b].reshape(C, HQ * WQ).T)
        tapidx_w = np.zeros((NSEG_PER_CORE, 2, 128, 64), np.int16)
        tapw_t = np.zeros((NSEG_PER_CORE, 128, 32), np.float32)
        sscale_t = np.zeros((NSEG_PER_CORE, 128, 8), np.float32)
        cellidx_w = np.zeros((NSEG_PER_CORE, 128, nch * 8), np.int16)
        nslotg = (SL + 7) // 8
        slotidx_w = np.zeros((NSEG_PER_CORE, nslotg, 128, 64), np.int16)
        for kk in range(NSEG_PER_CORE):
            tapidx, tapw, sscale, cellrow, slot_lists, _ = segs[(b, s0 + kk)]
            # pair-gather token i2 = j2*128 + p ; j2 = ch*2 + a ; pixel = ch*128+p
            tl = np.zeros((2048,), np.int32)
            for j2 in range(16):
                ch, a = j2 // 2, j2 % 2
                tl[j2 * 128:(j2 + 1) * 128] = tapidx[ch * 128:(ch + 1) * 128, a]
            for g in range(2):
                tapidx_w[kk, g] = _wrap16(tl[g * 1024:(g + 1) * 1024])
            tapw_t[kk] = tapw.reshape(8, 128, 4).transpose(1, 0, 2).reshape(128, 32)
            sscale_t[kk] = sscale.reshape(8, 128).T
            # slot gather: slot s in [off[k], off[k]+Lk[k]) for chunk k;
            # token j = s*128 + p gathers stage row = pixel (or NPIX zero row)
            sl = np.full((SL * 128,), NPIX, np.int32)
            for rank in range(nch * 128):
                k, p = rank // 128, rank % 128
                for l, pix in enumerate(slot_lists[rank]):
                    sl[(off[k] + l) * 128 + p] = pix
            for g in range(nslotg):
                seg_tok = sl[g * 1024:(g + 1) * 1024]
                w = _wrap16(seg_tok)
                slotidx_w[kk, g, :, :w.shape[1]] = w
            # final unique-row scatter: token i -> cell row
            cellidx_w[kk] = _wrap16(cellrow[:nch * 128])
        featT_pad = np.vstack([featT, np.zeros((1, C), np.float32)])
        in_maps.append({
            "featT": featT_pad,
            "tapidx": tapidx_w,
            "tapw": tapw_t,
            "sscale": sscale_t,
            "slotidx": slotidx_w,
            "cellidx": cellidx_w,
            "ident": np.eye(128, dtype=np.float32),
        })
    return in_maps, [int(x) for x in Lk], nch


# ----------------------------------------------------------------------------
# Device program (SPMD, static)
# ----------------------------------------------------------------------------

def build_nc(Lk, nch=NCHUNK, ablate=()):
    import concourse.bass as bass
    import concourse.bacc as bacc
    import concourse.tile as tile
    from concourse import mybir

    A = set(ablate)  # {'taps','comb','scatter','f2'} stages to skip (perf study)

    f32 = mybir.dt.float32
    i16 = mybir.dt.int16

    SL = int(sum(Lk))
    off = [0] * nch
    for k in range(1, nch):
        off[k] = off[k - 1] + Lk[k - 1]
    nslotg = (SL + 7) // 8

    nc = bacc.Bacc(None, target_bir_lowering=False)
    featT = nc.declare_dram_parameter("featT", [HQ * WQ + 1, C], f32,
                                      isOutput=False)
    tapidx = nc.declare_dram_parameter("tapidx", [NSEG_PER_CORE, 2, 128, 64],
                                       i16, isOutput=False)
    tapw = nc.declare_dram_parameter("tapw", [NSEG_PER_CORE, 128, 32], f32,
                                     isOutput=False)
    sscale = nc.declare_dram_parameter("sscale", [NSEG_PER_CORE, 128, 8], f32,
                                       isOutput=False)
    slotidx = nc.declare_dram_parameter(
        "slotidx", [NSEG_PER_CORE, nslotg, 128, 64], i16, isOutput=False)
    cellidx = nc.declare_dram_parameter("cellidx", [NSEG_PER_CORE, 128, nch * 8],
                                        i16, isOutput=False)
    identp = nc.declare_dram_parameter("ident", [128, 128], f32, isOutput=False)
    f2out = nc.declare_dram_parameter("f2out", [NSEG_PER_CORE, C, NPIX], f32,
                                      isOutput=True)
    gouts = [nc.declare_dram_parameter(f"gout{k}", [V, C], f32, isOutput=True)
             for k in range(NSEG_PER_CORE)]

    with tile.TileContext(nc) as tc:
        with (
            tc.tile_pool(name="const", bufs=1) as cpool,
            tc.tile_pool(name="work", bufs=WORK_BUFS) as pool,
            tc.tile_pool(name="psum", bufs=4, space="PSUM") as pp,
            tc.tile_pool(name="dram", bufs=1, space="DRAM") as dpool,
        ):
            stage0 = dpool.tile([NPIX + 1, C], f32, tag="stage0")
            stage1 = dpool.tile([NPIX + 1, C], f32, tag="stage1")
            stages = [stage0, stage1]
            ident = cpool.tile([128, 128], f32)
            nc.sync.dma_start(ident[:], identp[:])
            zrow = cpool.tile([1, C], f32)
            nc.vector.memset(zrow[:], 0.0)

            # Phase 1 (both segments): tap gather -> weighted combine -> scale
            # -> stage write. Phase 2 (both segments): slot gather -> cell
            # reduce -> scatter + f2 transposes. Issuing both segments'
            # front-ends before any back-end keeps the in-order Pool engine
            # from head-of-line blocking on segment 0's stage-write latency.
            seg = [{} for _ in range(NSEG_PER_CORE)]
            for k in range(NSEG_PER_CORE):
                if 'scatter' not in A: nc.sync.dma_start(stages[k][NPIX:NPIX + 1, :], zrow[:])

                tapw_t = pool.tile([128, 32], f32, tag="tapw")
                nc.sync.dma_start(tapw_t[:], tapw[k])
                sscale_t = pool.tile([128, 8], f32, tag="sscale")
                nc.sync.dma_start(sscale_t[:], sscale[k])

                feat_pairs = bass.AP(featT[:].tensor, 0,
                                     [[C, HQ * WQ], [1, 2 * C]])
                taps = pool.tile([128, 32, 128], f32, tag="taps")
                tmp = pool.tile([128, 32, 128], f32, tag="tmp")
                roiT = pool.tile([128, 8, 128], f32, tag="roiT")
                for g in range(2 if 'taps' not in A else 0):
                    tapidx_t = pool.tile([128, 64], i16, tag="tapidx")
                    nc.sync.dma_start(tapidx_t[:], tapidx[k, g])
                    gout_ap = (taps[:, g * 16:(g + 1) * 16, :]
                               .rearrange("p a c -> p (a c)")
                               .rearrange("p (j c) -> p j c", c=2 * C))
                    nc.gpsimd.dma_gather(
                        gout_ap, feat_pairs, tapidx_t[:],
                        num_idxs=MAX_GATHER, num_idxs_reg=MAX_GATHER,
                        elem_size=2 * C, elem_step=C,
                    )
                    if 'comb' not in A:
                        sl16 = slice(g * 16, (g + 1) * 16)
                        nc.vector.tensor_tensor(
                            out=tmp[:, sl16, :], in0=taps[:, sl16, :],
                            in1=tapw_t[:, sl16].to_broadcast([128, 16, 128]),
                            op=mybir.AluOpType.mult,
                        )
                        nc.vector.tensor_reduce(
                            out=roiT[:, g * 4:(g + 1) * 4, :],
                            in_=tmp[:, sl16, :].rearrange(
                                "p (ch t) c -> p ch c t", ch=4, t=4),
                            axis=mybir.AxisListType.X,
                            op=mybir.AluOpType.add,
                        )
                rsT = pool.tile([128, 8, 128], f32, tag="rsT")
                if 'comb' not in A: nc.vector.tensor_tensor(
                    out=rsT[:], in0=roiT[:],
                    in1=sscale_t[:].to_broadcast([128, 8, 128]),
                    op=mybir.AluOpType.mult,
                )
                # stage pixel rows to DRAM (row q = pixel q)
                if 'scatter' not in A: nc.sync.dma_start(
                    stages[k][0:NPIX, :].rearrange("(ch p) c -> p ch c", p=128),
                    rsT[:],
                )
                seg[k]["roiT"] = roiT
                # prefetch phase-2 index tables now (unique tags keep all live)
                cellidx_t = pool.tile([128, nch * 8], i16, tag=f"cellidx{k}")
                nc.sync.dma_start(cellidx_t[:], cellidx[k])
                seg[k]["cellidx"] = cellidx_t
                seg[k]["sidx"] = []
                for g in range(nslotg if 'scatter' not in A else 0):
                    sidx_t = pool.tile([128, 64], i16, tag=f"sidx{k}_{g}")
                    nc.sync.dma_start(sidx_t[:], slotidx[k, g])
                    seg[k]["sidx"].append(sidx_t)

            for k in range(NSEG_PER_CORE):
                roiT = seg[k]["roiT"]
                cellidx_t = seg[k]["cellidx"]
                slots = pool.tile([128, SL, 128], f32, tag="slots")
                for g in range(nslotg if 'scatter' not in A else 0):
                    ns = min(8, SL - g * 8)
                    sidx_t = seg[k]["sidx"][g]
                    nc.gpsimd.dma_gather(
                        slots[:, g * 8:g * 8 + ns, :], stages[k][:, :],
                        sidx_t[:, :ns * 8],
                        num_idxs=ns * 128, num_idxs_reg=ns * 128, elem_size=C,
                    )
                gsum = pool.tile([128, nch, 128], f32, tag="gsum")
                for ch in range(nch if 'scatter' not in A else 0):
                    nc.vector.tensor_reduce(
                        out=gsum[:, ch, :],
                        in_=slots[:, off[ch]:off[ch] + Lk[ch], :]
                            .rearrange("p l c -> p c l"),
                        axis=mybir.AxisListType.X,
                        op=mybir.AluOpType.add,
                    )
                if 'scatter' not in A: nc.gpsimd.dma_scatter_add(
                    gouts[k][:], gsum[:], cellidx_t[:],
                    num_idxs=nch * 128, num_idxs_reg=nch * 128, elem_size=C,
                )
                roi2d = pool.tile([128, NPIX], f32, tag="roi2d")
                for ch in range(8 if 'f2' not in A else 0):
                    ps = pp.tile([128, 128], f32, tag="tps")
                    nc.tensor.transpose(out=ps[:], in_=roiT[:, ch, :],
                                        identity=ident[:])
                    nc.scalar.copy(out=roi2d[:, ch * 128:(ch + 1) * 128],
                                   in_=ps[:])
                if 'f2' not in A: nc.sync.dma_start(f2out[k], roi2d[:])
    nc.finalize()
    return nc


_NC_CACHE = {}


def _get_nc(Lk, nch):
    key = (tuple(Lk), nch)
    if key not in _NC_CACHE:
        _NC_CACHE[key] = build_nc(Lk, nch)
    return _NC_CACHE[key]


# ----------------------------------------------------------------------------
# Entry point
# ----------------------------------------------------------------------------

def kernel(encoder_features, depths, intrinsics, masks, _trace=False):
    from concourse.bass_utils import run_bass_kernel_spmd

    in_maps, Lk, nch = build_tables(encoder_features, depths, intrinsics, masks)
    nc = _get_nc(Lk, nch)
    res = run_bass_kernel_spmd(nc, in_maps, core_ids=list(range(NCORES)),
                               trace=_trace)
    f2 = np.zeros((NB, NS, C, T, T), np.float32)
    g = np.zeros((NB, NS, T, T, T, C), np.float32)
    for core in range(NCORES):
        b = core // 4
        s0 = 2 * (core % 4)
        r = res.results[core]
        f2[b, s0:s0 + 2] = r["f2out"].reshape(NSEG_PER_CORE, C, T, T)
        for k in range(NSEG_PER_CORE):
            g[b, s0 + k] = r[f"gout{k}"].reshape(T, T, T, C)
    if _trace:
        kernel._last_result = res
    return f2, g


# revision 34
# speedup vs baseline: 1.1029x; 1.0122x over previous
"""CubeForwardProjection Trainium2 kernel.

Strategy: 16 (batch, segment) units over 8 NeuronCores, 2 segments per core
(both from the same batch). All data-dependent control (crop box, bilinear
coords, voxel assignment) is computed on host exactly mirroring the
reference's jnp float32 semantics and baked into per-core index/weight
tables. The device does all the heavy data-plane work per segment:

  1. dma_gather (2 x 1024 idx): fetch the 4 bilinear taps per output pixel
     as 2 adjacent-row PAIRS (overlapping 1KB elements, elem_step=128) from
     the pixel-major feature table in HBM.
  2. VectorE: weighted 4-tap reduction (crop+resize+mask all folded into the
     tap weights) -> roiT (pixel-major 1024 x 128 ROI features), sliced per
     gather group for pipelining.
  3. VectorE: scale rows by per-pixel scatter weight w/cnt (validity and
     scatter-mean divisor folded) -> RsT; staged to DRAM pixel-row-major.
  4. dma_gather slots: regroup pixel rows by voxel cell (cells sorted by
     count desc; per-128-cell chunk slot length = chunk max count), then
     VectorE segmented reduce -> per-cell sums.
  5. dma_scatter_add: one pass of UNIQUE cell rows into the (32768, 128)
     voxel grid (duplicate-row scatter_add loses updates - verified on HW -
     so duplicates are pre-combined in step 4; dummy rows carry zeros).
  6. TensorE transposes roiT -> channel-major roi2d, DMA'd out as the 2D ROI
     feature output.

The program is identical across cores (SPMD); all per-segment variation
lives in the table values; only the slot lengths Lk / active chunk count
(data-dependent, known at build time) parameterize the compiled shape.

Known HW constraints found while bringing this up (cost one device crash
each): dma_gather/dma_scatter_add num_idxs must be <= 1024, and
dma_scatter_add does NOT accumulate correctly across duplicate indices
within one instruction (RMW race between DMA engines).
"""
import numpy as np

for _p in ("/opt/trn_rl_repo",):
    import sys
    if _p not in sys.path:
        sys.path.insert(0, _p)

import jax
import jax.numpy as jnp

T = 32
HQ, WQ = 120, 160
NPIX = T * T          # 1024
C = 128
NCHUNK = 8            # 128-cell chunks
NSEG_PER_CORE = 2
NCORES = 8
NB, NS = 2, 8
V = T * T * T         # 32768
MAX_GATHER = 1024     # dma_gather num_idxs > 1024 crashes the exec unit
WORK_BUFS = 2         # working tile pool slots (overlap depth)

_CPU = jax.devices("cpu")[0]


# ----------------------------------------------------------------------------
# Host-side table construction (mirrors reference.py jnp f32 semantics)
# ----------------------------------------------------------------------------

def _crop_params(m_np):
    ys, xs = np.nonzero(m_np)
    y0, y1 = int(ys.min()), int(ys.max())
    x0, x1 = int(xs.min()), int(xs.max())
    side = max(y1 - y0, x1 - x0)
    cy, cx = (y0 + y1) // 2, (x0 + x1) // 2
    ny0 = max(cy - side // 2, 0)
    nx0 = max(cx - side // 2, 0)
    return ny0, nx0, side


def _bilinear_coords(n_in, size=T):
    x = (jnp.arange(size, dtype=jnp.float32) + 0.5) * (n_in / size) - 0.5
    x = jnp.clip(x, 0.0, n_in - 1.0)
    lo = jnp.floor(x).astype(jnp.int32)
    hi = jnp.minimum(lo + 1, n_in - 1)
    return np.asarray(lo), np.asarray(hi), np.asarray(x - lo)


def _nearest_coords(n_in, size=T):
    return np.asarray(jnp.floor(jnp.arange(size) * (n_in / size)).astype(jnp.int32))


def _project(depth, Kinv):
    H, W = depth.shape
    xs, ys = jnp.meshgrid(jnp.arange(W, dtype=depth.dtype),
                          jnp.arange(H, dtype=depth.dtype), indexing='xy')
    pix = jnp.stack([xs, ys, jnp.ones_like(xs)], axis=-1)
    cam = pix @ Kinv.T
    return cam / cam[..., 2:3] * depth[..., None]


def _segment_tables(points_np, m_np):
    """tapidx (1024,4) i32, tapw (1024,4) f32, sscale (1024,) f32,
    cellrow (1024,) i32 for one valid segment."""
    ny0, nx0, side = _crop_params(m_np)
    ylo, yhi, fy = _bilinear_coords(side)
    xlo, xhi, fx = _bilinear_coords(side)

    # pair taps: one gathered element = feat rows (y_a, x0) and (y_a, x0+1).
    # tapidx[q, a] = row of (y_a, x0); tapw[q, a*2+h] = weight of half h.
    tapidx = np.zeros((NPIX, 2), np.int32)
    tapw = np.zeros((NPIX, 4), np.float32)
    ys_abs = np.stack([ny0 + ylo, ny0 + yhi])
    wy = np.stack([1.0 - fy, fy]).astype(np.float32)
    wx = np.stack([1.0 - fx, fx]).astype(np.float32)
    x0_abs = nx0 + xlo
    x1_abs = nx0 + xhi
    m_at = lambda yy, xx: m_np[yy, xx].astype(np.float32)
    for a in range(2):
        for ty in range(T):
            y = int(ys_abs[a][ty])
            wya = wy[a][ty]
            if y >= HQ:
                continue
            for tx in range(T):
                q = ty * T + tx
                x0 = int(x0_abs[tx]); x1 = int(x1_abs[tx])
                w0 = w1 = 0.0
                idx = 0
                if x0 < WQ:
                    idx = y * WQ + x0
                    w0 = wya * wx[0][tx] * m_at(y, x0)
                    if x1 == x0:
                        w0 += wya * wx[1][tx] * m_at(y, x0)
                    elif x1 < WQ:
                        w1 = wya * wx[1][tx] * m_at(y, x1)
                tapidx[q, a] = idx
                tapw[q, a * 2 + 0] = w0
                tapw[q, a * 2 + 1] = w1

    # points path (mirror reference)
    yi = ny0 + _nearest_coords(side)
    xi = nx0 + _nearest_coords(side)
    mp = points_np * m_np[..., None].astype(points_np.dtype)
    ptile = np.zeros((T, T, 3), np.float32)
    oky, okx = yi < HQ, xi < WQ
    ptile[np.ix_(oky, okx)] = mp[yi[oky]][:, xi[okx]]
    ptsj = jnp.asarray(ptile.reshape(-1, 3))
    valid = jnp.all(ptsj != 0, axis=1)
    big = jnp.float32(1e30)
    mn = jnp.min(jnp.where(valid[:, None], ptsj, big), axis=0)
    mx = jnp.max(jnp.where(valid[:, None], ptsj, -big), axis=0)
    center = (mn + mx) * 0.5
    size = jnp.max(mx - mn)
    p = (ptsj - center) / size + 0.5
    inb = jnp.all((p >= 0) & (p <= 1), axis=1)
    w = np.asarray(valid & inb)
    gi = jnp.clip((p * (T - 1)).astype(jnp.int32), 0, T - 1)
    flat = np.asarray(gi[:, 0] * T * T + gi[:, 1] * T + gi[:, 2])

    act = np.nonzero(w)[0]
    cells, inv, cnt = np.unique(flat[act], return_inverse=True,
                                return_counts=True)
    sscale = np.zeros((NPIX,), np.float32)
    sscale[act] = 1.0 / cnt[inv].astype(np.float32)

    # scatter plan: cells sorted by count desc; per 128-cell chunk k the slot
    # length is the chunk's max count; cell (k,p) sums its pixels from slots.
    order = np.argsort(-cnt, kind='stable')
    n_u = len(cells)
    cellrow = np.full((NPIX,), -1, np.int32)       # -1 -> dummy (unoccupied)
    chunk_cnt = np.zeros((NCHUNK,), np.int32)      # per-chunk max count
    # slotpix[u_slot_position] lists: build per (chunk, partition)
    slot_lists = [[] for _ in range(NPIX)]         # per sorted-cell pixel lists
    pix_of_cell = [[] for _ in range(n_u)]
    for j in np.argsort(inv, kind='stable'):
        pix_of_cell[inv[j]].append(int(act[j]))
    for rank, ci in enumerate(order):
        cellrow[rank] = cells[ci]
        slot_lists[rank] = pix_of_cell[ci]
        k = rank // 128
        chunk_cnt[k] = max(chunk_cnt[k], cnt[ci])
    occupied = set(cells.tolist())
    dummy = next(i for i in range(NPIX + 1) if i not in occupied)
    cellrow[cellrow < 0] = dummy
    return tapidx, tapw, sscale, cellrow, slot_lists, chunk_cnt


def _empty_tables():
    return (np.zeros((NPIX, 2), np.int32), np.zeros((NPIX, 4), np.float32),
            np.zeros((NPIX,), np.float32), np.zeros((NPIX,), np.int32),
            [[] for _ in range(NPIX)], np.zeros((NCHUNK,), np.int32))


def _wrap16(idx_logical):
    """Logical gather order -> (128, n//16) int16 wrapped/replicated layout."""
    n = idx_logical.shape[0]
    a = idx_logical.reshape(n // 16, 16).T.astype(np.int16)   # (16, n//16)
    return np.ascontiguousarray(np.tile(a, (8, 1)))


def build_tables(encoder_features, depths, intrinsics, masks):
    """Returns (per-core input maps, Lk) where Lk[k] is the global slot
    length of cell chunk k."""
    ef = np.ascontiguousarray(np.asarray(encoder_features, dtype=np.float32))
    depths_np = np.asarray(depths, dtype=np.float32)
    masks_np = np.asarray(masks)
    with jax.default_device(_CPU):
        pts = {}
        for b in range(NB):
            K = jnp.asarray(np.asarray(intrinsics[b], dtype=np.float32))
            K = K.at[2, 2].multiply(4.0)
            Kinv = jnp.linalg.inv(K)
            pts[b] = np.asarray(_project(jnp.asarray(depths_np[b, ::4, ::4]),
                                         Kinv))
        segs = {}
        for b in range(NB):
            fd = depths_np[b, ::4, ::4]
            for s in range(NS):
                m = masks_np[b, s, ::4, ::4] & (fd > 0)
                if m.sum() < 10:
                    segs[(b, s)] = _empty_tables()
                else:
                    segs[(b, s)] = _segment_tables(pts[b], m)

    # global per-chunk slot lengths + active chunk count
    Lk = np.ones((NCHUNK,), np.int64)
    nch = 1
    for t in segs.values():
        Lk = np.maximum(Lk, np.asarray(t[5], np.int64))
        n_u = sum(len(x) > 0 for x in t[4])
        nch = max(nch, (n_u + 127) // 128)
    Lk = Lk[:nch]
    SL = int(Lk.sum())
    off = np.zeros((nch,), np.int64)
    off[1:] = np.cumsum(Lk)[:-1]

    in_maps = []
    for core in range(NCORES):
        b = core // 4
        s0 = 2 * (core % 4)
        featT = np.ascontiguousarray(ef[b].reshape(C, HQ * WQ).T)
        tapidx_w = np.zeros((NSEG_PER_CORE, 2, 128, 64), np.int16)
        tapw_t = np.zeros((NSEG_PER_CORE, 128, 32), np.float32)
        sscale_t = np.zeros((NSEG_PER_CORE, 128, 8), np.float32)
        cellidx_w = np.zeros((NSEG_PER_CORE, 128, nch * 8), np.int16)
        nslotg = (SL + 7) // 8
        slotidx_w = np.zeros((NSEG_PER_CORE, nslotg, 128, 64), np.int16)
        for kk in range(NSEG_PER_CORE):
            tapidx, tapw, sscale, cellrow, slot_lists, _ = segs[(b, s0 + kk)]
            # pair-gather token i2 = j2*128 + p ; j2 = ch*2 + a ; pixel = ch*128+p
            tl = np.zeros((2048,), np.int32)
            for j2 in range(16):
                ch, a = j2 // 2, j2 % 2
                tl[j2 * 128:(j2 + 1) * 128] = tapidx[ch * 128:(ch + 1) * 128, a]
            for g in range(2):
                tapidx_w[kk, g] = _wrap16(tl[g * 1024:(g + 1) * 1024])
            tapw_t[kk] = tapw.reshape(8, 128, 4).transpose(1, 0, 2).reshape(128, 32)
            sscale_t[kk] = sscale.reshape(8, 128).T
            # slot gather: slot s in [off[k], off[k]+Lk[k]) for chunk k;
            # token j = s*128 + p gathers stage row = pixel (or NPIX zero row)
            sl = np.full((SL * 128,), NPIX, np.int32)
            for rank in range(nch * 128):
                k, p = rank // 128, rank % 128
                for l, pix in enumerate(slot_lists[rank]):
                    sl[(off[k] + l) * 128 + p] = pix
            for g in range(nslotg):
                seg_tok = sl[g * 1024:(g + 1) * 1024]
                w = _wrap16(seg_tok)
                slotidx_w[kk, g, :, :w.shape[1]] = w
            # final unique-row scatter: token i -> cell row
            cellidx_w[kk] = _wrap16(cellrow[:nch * 128])
        featT_pad = np.vstack([featT, np.zeros((1, C), np.float32)])
        in_maps.append({
            "featT": featT_pad,
            "tapidx": tapidx_w,
            "tapw": tapw_t,
            "sscale": sscale_t,
            "slotidx": slotidx_w,
            "cellidx": cellidx_w,
            "ident": np.eye(128, dtype=np.float32),
        })
    return in_maps, [int(x) for x in Lk], nch


# ----------------------------------------------------------------------------
# Device program (SPMD, static)
# ----------------------------------------------------------------------------

def build_nc(Lk, nch=NCHUNK, ablate=()):
    import concourse.bass as bass
    import concourse.bacc as bacc
    import concourse.tile as tile
    from concourse import mybir

    A = set(ablate)  # {'taps','comb','scatter','f2'} stages to skip (perf study)

    f32 = mybir.dt.float32
    i16 = mybir.dt.int16

    SL = int(sum(Lk))
    off = [0] * nch
    for k in range(1, nch):
        off[k] = off[k - 1] + Lk[k - 1]
    nslotg = (SL + 7) // 8

    nc = bacc.Bacc(None, target_bir_lowering=False)
    featT = nc.declare_dram_parameter("featT", [HQ * WQ + 1, C], f32,
                                      isOutput=False)
    tapidx = nc.declare_dram_parameter("tapidx", [NSEG_PER_CORE, 2, 128, 64],
                                       i16, isOutput=False)
    tapw = nc.declare_dram_parameter("tapw", [NSEG_PER_CORE, 128, 32], f32,
                                     isOutput=False)
    sscale = nc.declare_dram_parameter("sscale", [NSEG_PER_CORE, 128, 8], f32,
                                       isOutput=False)
    slotidx = nc.declare_dram_parameter(
        "slotidx", [NSEG_PER_CORE, nslotg, 128, 64], i16, isOutput=False)
    cellidx = nc.declare_dram_parameter("cellidx", [NSEG_PER_CORE, 128, nch * 8],
                                        i16, isOutput=False)
    identp = nc.declare_dram_parameter("ident", [128, 128], f32, isOutput=False)
    f2out = nc.declare_dram_parameter("f2out", [NSEG_PER_CORE, C, NPIX], f32,
                                      isOutput=True)
    gouts = [nc.declare_dram_parameter(f"gout{k}", [V, C], f32, isOutput=True)
             for k in range(NSEG_PER_CORE)]

    with tile.TileContext(nc) as tc:
        with (
            tc.tile_pool(name="const", bufs=1) as cpool,
            tc.tile_pool(name="work", bufs=WORK_BUFS) as pool,
            tc.tile_pool(name="psum", bufs=4, space="PSUM") as pp,
            tc.tile_pool(name="dram", bufs=1, space="DRAM") as dpool,
        ):
            stage0 = dpool.tile([NPIX + 1, C], f32, tag="stage0")
            stage1 = dpool.tile([NPIX + 1, C], f32, tag="stage1")
            stages = [stage0, stage1]
            ident = cpool.tile([128, 128], f32)
            nc.sync.dma_start(ident[:], identp[:])
            zrow = cpool.tile([1, C], f32)
            nc.vector.memset(zrow[:], 0.0)

            # Phase 1 (both segments): tap gather -> weighted combine -> scale
            # -> stage write. Phase 2 (both segments): slot gather -> cell
            # reduce -> scatter + f2 transposes. Issuing both segments'
            # front-ends before any back-end keeps the in-order Pool engine
            # from head-of-line blocking on segment 0's stage-write latency.
            seg = [{} for _ in range(NSEG_PER_CORE)]
            for k in range(NSEG_PER_CORE):
                if 'scatter' not in A: nc.sync.dma_start(stages[k][NPIX:NPIX + 1, :], zrow[:])

                tapw_t = pool.tile([128, 32], f32, tag="tapw")
                nc.sync.dma_start(tapw_t[:], tapw[k])
                sscale_t = pool.tile([128, 8], f32, tag="sscale")
                nc.sync.dma_start(sscale_t[:], sscale[k])

                feat_pairs = bass.AP(featT[:].tensor, 0,
                                     [[C, HQ * WQ], [1, 2 * C]])
                taps = pool.tile([128, 32, 128], f32, tag="taps")
                tmp = pool.tile([128, 32, 128], f32, tag="tmp")
                roiT = pool.tile([128, 8, 128], f32, tag="roiT")
                for g in range(2 if 'taps' not in A else 0):
                    tapidx_t = pool.tile([128, 64], i16, tag="tapidx")
                    nc.sync.dma_start(tapidx_t[:], tapidx[k, g])
                    gout_ap = (taps[:, g * 16:(g + 1) * 16, :]
                               .rearrange("p a c -> p (a c)")
                               .rearrange("p (j c) -> p j c", c=2 * C))
                    nc.gpsimd.dma_gather(
                        gout_ap, feat_pairs, tapidx_t[:],
                        num_idxs=MAX_GATHER, num_idxs_reg=MAX_GATHER,
                        elem_size=2 * C, elem_step=C,
                    )
                    if 'comb' not in A:
                        sl16 = slice(g * 16, (g + 1) * 16)
                        nc.vector.tensor_tensor(
                            out=tmp[:, sl16, :], in0=taps[:, sl16, :],
                            in1=tapw_t[:, sl16].to_broadcast([128, 16, 128]),
                            op=mybir.AluOpType.mult,
                        )
                        nc.vector.tensor_reduce(
                            out=roiT[:, g * 4:(g + 1) * 4, :],
                            in_=tmp[:, sl16, :].rearrange(
                                "p (ch t) c -> p ch c t", ch=4, t=4),
                            axis=mybir.AxisListType.X,
                            op=mybir.AluOpType.add,
                        )
                rsT = pool.tile([128, 8, 128], f32, tag="rsT")
                if 'comb' not in A: nc.vector.tensor_tensor(
                    out=rsT[:], in0=roiT[:],
                    in1=sscale_t[:].to_broadcast([128, 8, 128]),
                    op=mybir.AluOpType.mult,
                )
                # stage pixel rows to DRAM (row q = pixel q)
                if 'scatter' not in A: nc.sync.dma_start(
                    stages[k][0:NPIX, :].rearrange("(ch p) c -> p ch c", p=128),
                    rsT[:],
                )
                seg[k]["roiT"] = roiT
                # prefetch phase-2 index tables now (unique tags keep all live)
                cellidx_t = pool.tile([128, nch * 8], i16, tag=f"cellidx{k}")
                nc.sync.dma_start(cellidx_t[:], cellidx[k])
                seg[k]["cellidx"] = cellidx_t
                seg[k]["sidx"] = []
                for g in range(nslotg if 'scatter' not in A else 0):
                    sidx_t = pool.tile([128, 64], i16, tag=f"sidx{k}_{g}")
                    nc.sync.dma_start(sidx_t[:], slotidx[k, g])
                    seg[k]["sidx"].append(sidx_t)

            for k in range(NSEG_PER_CORE):
                roiT = seg[k]["roiT"]
                cellidx_t = seg[k]["cellidx"]
                slots = pool.tile([128, SL, 128], f32, tag="slots")
                for g in range(nslotg if 'scatter' not in A else 0):
                    ns = min(8, SL - g * 8)
                    sidx_t = seg[k]["sidx"][g]
                    nc.gpsimd.dma_gather(
                        slots[:, g * 8:g * 8 + ns, :], stages[k][:, :],
                        sidx_t[:, :ns * 8],
                        num_idxs=ns * 128, num_idxs_reg=ns * 128, elem_size=C,
                    )
                gsum = pool.tile([128, nch, 128], f32, tag="gsum")
                ch = 0
                while ch < (nch if 'scatter' not in A else 0):
                    if Lk[ch] == 1:
                        # run of single-slot chunks: plain copy, on ScalarE to
                        # keep VectorE free
                        j = ch
                        while j < nch and Lk[j] == 1:
                            j += 1
                        nc.scalar.copy(
                            out=gsum[:, ch:j, :],
                            in_=slots[:, off[ch]:off[ch] + (j - ch), :],
                        )
                        ch = j
                    else:
                        nc.vector.tensor_reduce(
                            out=gsum[:, ch, :],
                            in_=slots[:, off[ch]:off[ch] + Lk[ch], :]
                                .rearrange("p l c -> p c l"),
                            axis=mybir.AxisListType.X,
                            op=mybir.AluOpType.add,
                        )
                        ch += 1
                if 'scatter' not in A: nc.gpsimd.dma_scatter_add(
                    gouts[k][:], gsum[:], cellidx_t[:],
                    num_idxs=nch * 128, num_idxs_reg=nch * 128, elem_size=C,
                )
                roi2d = pool.tile([128, NPIX], f32, tag="roi2d")
                for ch in range(8 if 'f2' not in A else 0):
                    ps = pp.tile([128, 128], f32, tag="tps")
                    nc.tensor.transpose(out=ps[:], in_=roiT[:, ch, :],
                                        identity=ident[:])
                    nc.scalar.copy(out=roi2d[:, ch * 128:(ch + 1) * 128],
                                   in_=ps[:])
                if 'f2' not in A: nc.sync.dma_start(f2out[k], roi2d[:])
    nc.finalize()
    return nc


_NC_CACHE = {}


def _get_nc(Lk, nch):
    key = (tuple(Lk), nch)
    if key not in _NC_CACHE:
        _NC_CACHE[key] = build_nc(Lk, nch)
    return _NC_CACHE[key]


# ----------------------------------------------------------------------------
# Entry point
# ----------------------------------------------------------------------------

def kernel(encoder_features, depths, intrinsics, masks, _trace=False):
    from concourse.bass_utils import run_bass_kernel_spmd

    in_maps, Lk, nch = build_tables(encoder_features, depths, intrinsics, masks)
    nc = _get_nc(Lk, nch)
    res = run_bass_kernel_spmd(nc, in_maps, core_ids=list(range(NCORES)),
                               trace=_trace)
    f2 = np.zeros((NB, NS, C, T, T), np.float32)
    g = np.zeros((NB, NS, T, T, T, C), np.float32)
    for core in range(NCORES):
        b = core // 4
        s0 = 2 * (core % 4)
        r = res.results[core]
        f2[b, s0:s0 + 2] = r["f2out"].reshape(NSEG_PER_CORE, C, T, T)
        for k in range(NSEG_PER_CORE):
            g[b, s0 + k] = r[f"gout{k}"].reshape(T, T, T, C)
    if _trace:
        kernel._last_result = res
    return f2, g
